# revision 1
# baseline (speedup 1.0000x reference)
"""GRU single-step kernel for Trainium2, data-parallel over 8 NeuronCores.

Computes h_next = GRUCell(x, h_prev) with PyTorch gate layout [r; z; n]:
    gi = x @ W_ih.T + b_ih ; gh = h @ W_hh.T + b_hh
    r = sigmoid(gi_r + gh_r); z = sigmoid(gi_z + gh_z)
    n = tanh(gi_n + r * gh_n); h' = (1-z)*n + z*h

Strategy: shard batch (16384 -> 8 x 2048). Weights replicated, pre-transposed
and bf16-cast on host so they stream as the matmul moving operand straight
from SBUF. Activations pre-transposed on host into the PE-stationary layout
(feature dim on partitions), so the device does zero transposes. PSUM holds
r/z/n_i/n_h pre-activations per 128-row x 512-col half-tile (4 banks, double
buffered = all 8 banks). Epilogue on DVE/ACT; tanh(x) = 2*sigmoid(2x)-1 so the
ACT engine never reloads its function table.
"""

import os
import sys

import numpy as np

if "/opt/trn_rl_repo" not in sys.path:
    sys.path.insert(0, "/opt/trn_rl_repo")

H = 1024           # hidden == input size
B = 16384
NCORES = 8
BLOC = B // NCORES  # 2048 rows per core
P = 128
NTILES = BLOC // P  # 16 row tiles per core
KC = H // P         # 8 contraction chunks
NG = 512            # matmul free dim / PSUM bank width (fp32)

_cache = {}


def _build_program():
    from concourse import bacc, bass, mybir, tile

    f32 = mybir.dt.float32
    bf16 = mybir.dt.bfloat16
    Alu = mybir.AluOpType
    ActFn = mybir.ActivationFunctionType

    nc = bacc.Bacc("TRN2", target_bir_lowering=False, debug=False)

    # DRAM parameters (per-core shapes)
    xT = nc.declare_dram_parameter("xT", [NTILES, P, H], bf16, isOutput=False)
    hT = nc.declare_dram_parameter("hT", [NTILES, P, H], bf16, isOutput=False)
    hN = nc.declare_dram_parameter("hN", [NTILES, P, H], f32, isOutput=False)
    # wT[m*KC+kc] : [P, 3H] slice of W_m.T   (m=0 -> ih, m=1 -> hh)
    wT = nc.declare_dram_parameter("wT", [2 * KC, P, 3 * H], bf16, isOutput=False)
    # bias_b : broadcast biases [P, 4H] = [r_comb | z_comb | n_i | n_h]
    bias_b = nc.declare_dram_parameter("bias_b", [P, 4 * H], f32, isOutput=False)
    out = nc.declare_dram_parameter("h_next", [NTILES, P, H], f32, isOutput=True)

    with tile.TileContext(nc) as tc:
        with (
            tc.tile_pool(name="wpool", bufs=1) as wpool,
            tc.tile_pool(name="stream", bufs=3) as stream,
            tc.tile_pool(name="temps", bufs=2) as temps,
            tc.tile_pool(name="psum", bufs=2, space="PSUM") as psum,
        ):
            # Startup DMAs are chunked and emitted in the order the PE will
            # consume them, so the first matmul can start within a few us and
            # no single fat transfer serializes one DMA queue.
            w_tiles = [wpool.tile([P, 3 * H], bf16, tag=f"w{i}", name=f"w{i}") for i in range(2 * KC)]
            bias_t = wpool.tile([P, 4 * H], f32, tag="bias")

            # tile 0 stationary operands first (32KB per chunk)
            xt0 = stream.tile([P, KC, P], bf16, tag="xt")
            ht0 = stream.tile([P, KC, P], bf16, tag="ht")
            for kc in range(KC):
                nc.gpsimd.dma_start(out=xt0[:, kc, :], in_=xT[0, :, kc * P:(kc + 1) * P])
            for kc in range(KC):
                nc.gpsimd.dma_start(out=ht0[:, kc, :], in_=hT[0, :, kc * P:(kc + 1) * P])
            # weight chunks for half A (cols j*512 with j in 0,2,4), x-side
            # weights before h-side, then bias A, hn0, then the B halves.
            for m in range(2):
                for kc in range(KC):
                    for j in (0, 2, 4):
                        nc.sync.dma_start(
                            out=w_tiles[m * KC + kc][:, j * NG:(j + 1) * NG],
                            in_=wT[m * KC + kc, :, j * NG:(j + 1) * NG])
            for j in (0, 2, 4, 6):
                nc.sync.dma_start(out=bias_t[:, j * NG:(j + 1) * NG],
                                  in_=bias_b[:, j * NG:(j + 1) * NG])
            hn0 = stream.tile([P, H], f32, tag="hn")
            for j in range(2):
                nc.gpsimd.dma_start(out=hn0[:, j * NG:(j + 1) * NG],
                                  in_=hN[0, :, j * NG:(j + 1) * NG])
            for m in range(2):
                for kc in range(KC):
                    for j in (1, 3, 5):
                        nc.sync.dma_start(
                            out=w_tiles[m * KC + kc][:, j * NG:(j + 1) * NG],
                            in_=wT[m * KC + kc, :, j * NG:(j + 1) * NG])
            for j in (1, 3, 5, 7):
                nc.sync.dma_start(out=bias_t[:, j * NG:(j + 1) * NG],
                                  in_=bias_b[:, j * NG:(j + 1) * NG])

            for i in range(NTILES):
                if i == 0:
                    xt, ht, hn = xt0, ht0, hn0
                else:
                    xt = stream.tile([P, KC, P], bf16, tag="xt")
                    nc.gpsimd.dma_start(out=xt[:], in_=xT[i])
                    ht = stream.tile([P, KC, P], bf16, tag="ht")
                    nc.gpsimd.dma_start(out=ht[:], in_=hT[i])
                    hn = stream.tile([P, H], f32, tag="hn")
                    nc.gpsimd.dma_start(out=hn[:], in_=hN[i])
                ot = stream.tile([P, H], f32, tag="ot")

                for half in range(2):
                    g0 = half * NG
                    R = psum.tile([P, NG], f32, tag="R")
                    Z = psum.tile([P, NG], f32, tag="Z")
                    NI = psum.tile([P, NG], f32, tag="NI")
                    NH_ = psum.tile([P, NG], f32, tag="NH")

                    # x-side: gi chunks (r, z, n_i)
                    for kc in range(KC):
                        st = xt[:, kc, :]
                        w = w_tiles[kc]
                        nc.tensor.matmul(R[:], st, w[:, g0:g0 + NG],
                                         start=(kc == 0), stop=False)
                        nc.tensor.matmul(Z[:], st, w[:, H + g0:H + g0 + NG],
                                         start=(kc == 0), stop=False)
                        nc.tensor.matmul(NI[:], st, w[:, 2 * H + g0:2 * H + g0 + NG],
                                         start=(kc == 0), stop=(kc == KC - 1))
                    # h-side: gh chunks (r, z accumulate; n_h separate)
                    for kc in range(KC):
                        st = ht[:, kc, :]
                        w = w_tiles[KC + kc]
                        nc.tensor.matmul(R[:], st, w[:, g0:g0 + NG],
                                         start=False, stop=(kc == KC - 1))
                        nc.tensor.matmul(Z[:], st, w[:, H + g0:H + g0 + NG],
                                         start=False, stop=(kc == KC - 1))
                        nc.tensor.matmul(NH_[:], st, w[:, 2 * H + g0:2 * H + g0 + NG],
                                         start=(kc == 0), stop=(kc == KC - 1))

                    # epilogue for this [128, 512] half
                    rpre = temps.tile([P, NG], f32, tag="rpre")
                    nc.vector.tensor_tensor(rpre[:], R[:], bias_t[:, g0:g0 + NG], Alu.add)
                    r = temps.tile([P, NG], f32, tag="r")
                    nc.scalar.activation(r[:], rpre[:], ActFn.Sigmoid)

                    zpre = temps.tile([P, NG], f32, tag="zpre")
                    nc.vector.tensor_tensor(zpre[:], Z[:], bias_t[:, H + g0:H + g0 + NG], Alu.add)
                    z = temps.tile([P, NG], f32, tag="z")
                    nc.scalar.activation(z[:], zpre[:], ActFn.Sigmoid)

                    u = temps.tile([P, NG], f32, tag="u")
                    nc.vector.tensor_tensor(u[:], NH_[:], bias_t[:, 3 * H + g0:3 * H + g0 + NG], Alu.add)
                    t = temps.tile([P, NG], f32, tag="t")
                    nc.vector.tensor_tensor(t[:], r[:], u[:], Alu.mult)
                    v = temps.tile([P, NG], f32, tag="v")
                    nc.vector.tensor_tensor(v[:], NI[:], bias_t[:, 2 * H + g0:2 * H + g0 + NG], Alu.add)
                    npre = temps.tile([P, NG], f32, tag="npre")
                    nc.vector.tensor_tensor(npre[:], v[:], t[:], Alu.add)

                    # n = tanh(npre) = 2*sigmoid(2*npre) - 1 (single ACT table)
                    s = temps.tile([P, NG], f32, tag="s")
                    nc.scalar.activation(s[:], npre[:], ActFn.Sigmoid, scale=2.0)
                    n = temps.tile([P, NG], f32, tag="n")
                    nc.vector.tensor_scalar(n[:], s[:], 2.0, -1.0, Alu.mult, Alu.add)

                    # h' = n + z*(h - n)
                    hm1 = temps.tile([P, NG], f32, tag="hm1")
                    nc.vector.tensor_tensor(hm1[:], hn[:, g0:g0 + NG], n[:], Alu.subtract)
                    hm2 = temps.tile([P, NG], f32, tag="hm2")
                    nc.vector.tensor_tensor(hm2[:], z[:], hm1[:], Alu.mult)
                    nc.vector.tensor_tensor(ot[:, g0:g0 + NG], n[:], hm2[:], Alu.add)
                    nc.sync.dma_start(out=out[i, :, g0:g0 + NG], in_=ot[:, g0:g0 + NG])

    nc.compile()
    return nc


def _prep_inputs(x, h_prev, weight_ih, weight_hh, bias_ih, bias_hh):
    import ml_dtypes

    bf16 = ml_dtypes.bfloat16

    # activations -> [core, tile, p, kc, b] with value a[core*2048 + tile*128 + b, kc*128 + p]
    def to_stationary(a):
        v = a.reshape(NCORES, NTILES, P, KC, P).transpose(0, 1, 4, 3, 2)
        return np.ascontiguousarray(v).astype(bf16).reshape(NCORES, NTILES, P, H)

    xT = to_stationary(x)
    hT = to_stationary(h_prev)
    hN = np.ascontiguousarray(h_prev.reshape(NCORES, NTILES, P, H)).astype(np.float32)

    # weights -> W.T chunked: [m*KC+kc, p, g] = W_m[g, kc*128+p]
    def wt_chunks(w):
        return np.ascontiguousarray(w.T.reshape(KC, P, 3 * H)).astype(bf16)

    wT = np.concatenate([wt_chunks(weight_ih), wt_chunks(weight_hh)], axis=0)

    b_r = bias_ih[:H] + bias_hh[:H]
    b_z = bias_ih[H:2 * H] + bias_hh[H:2 * H]
    b_ni = bias_ih[2 * H:]
    b_nh = bias_hh[2 * H:]
    bias_vec = np.concatenate([b_r, b_z, b_ni, b_nh]).astype(np.float32)
    bias_b = np.ascontiguousarray(np.broadcast_to(bias_vec, (P, 4 * H)))

    in_maps = []
    for c in range(NCORES):
        in_maps.append({
            "xT": xT[c], "hT": hT[c], "hN": hN[c],
            "wT": wT, "bias_b": bias_b,
        })
    return in_maps


def kernel(x, h_prev, weight_ih, weight_hh, bias_ih, bias_hh):
    from concourse.bass_utils import run_bass_kernel_spmd

    x = np.asarray(x, dtype=np.float32)
    h_prev = np.asarray(h_prev, dtype=np.float32)
    weight_ih = np.asarray(weight_ih, dtype=np.float32)
    weight_hh = np.asarray(weight_hh, dtype=np.float32)
    bias_ih = np.asarray(bias_ih, dtype=np.float32)
    bias_hh = np.asarray(bias_hh, dtype=np.float32)

    if "nc" not in _cache:
        _cache["nc"] = _build_program()
    nc = _cache["nc"]

    in_maps = _prep_inputs(x, h_prev, weight_ih, weight_hh, bias_ih, bias_hh)
    trace = os.environ.get("GRU_TRACE", "0") == "1"
    res = run_bass_kernel_spmd(nc, in_maps, list(range(NCORES)), trace=trace)
    kernel._last_exec_ns = res.exec_time_ns

    outs = [np.asarray(res.results[c]["h_next"]).reshape(BLOC, H) for c in range(NCORES)]
    return np.concatenate(outs, axis=0).astype(np.float32)


kernel._last_exec_ns = None



# revision 2
# speedup vs baseline: 1.4584x; 1.4584x over previous
"""GRU single-step kernel for Trainium2, data-parallel over 8 NeuronCores.

Computes h_next = GRUCell(x, h_prev) with PyTorch gate layout [r; z; n]:
    gi = x @ W_ih.T + b_ih ; gh = h @ W_hh.T + b_hh
    r = sigmoid(gi_r + gh_r); z = sigmoid(gi_z + gh_z)
    n = tanh(gi_n + r * gh_n); h' = (1-z)*n + z*h

Strategy: shard batch (16384 -> 8 x 2048); weights replicated. All matmuls in
fp8(e4m3) with DoubleRow perf mode (2 k-subtiles per pass, 0.5 cyc/row = 4x
bf16 MAC rate). Weight-stationary layout: gates on PSUM partitions, batch on
the free dim, so the per-gate biases fold into the ACT engine's per-partition
bias operand for free. Weights are pre-scaled by 32 so fp8 quantization stays
out of the subnormal range; the 1/32 folds into the ACT scale operand.

fp8 error compensation (kills the dominant n-gate x-side error): the PSUM
group for n's gi accumulates three fp8 passes at identical scale,
  xq @ Wq  +  dx @ Wq  +  xq @ dW,
where dx = fp8(x - xq) and dW = fp8(32W - Wq) are unscaled fp8 residuals
(subnormals cover the small range). Host-side sim: rel err 1.4e-2 vs 2e-2.

Epilogue per (block, j) on [128 gates, 512 batch] tiles:
  ACT : r = sig(R/32 + br), z = sig(Z/32 + bz), n = tanh(NI'/32 + bni)
  DVE : t = (NH + 32*bnh)*r  (fused scalar_tensor_tensor), NI' = NI + t
        (in-place PSUM), hm1 = h - n, out = n + hm2
  POOL: hm2 = z*hm1, plus all activation/output DMA issue (cheap SWDGE)
All sigmoid/tanh live in one ACT table -> no table reloads.
"""

import os
import sys

import numpy as np

if "/opt/trn_rl_repo" not in sys.path:
    sys.path.insert(0, "/opt/trn_rl_repo")

H = 1024            # hidden == input size
B = 16384
NCORES = 8
BLOC = B // NCORES  # 2048 rows per core
NB = 512            # batch columns per block (PSUM bank width)
NBLK = BLOC // NB   # 4 blocks per core
KP = 4              # k-pairs (DoubleRow consumes 2x128 contraction per pass)
NJ = H // 128       # 8 hidden chunks of 128 gates
S = 32.0            # weight pre-scale

_cache = {}


def _build_program():
    from concourse import bacc, bass, mybir, tile

    f32 = mybir.dt.float32
    bf16 = mybir.dt.bfloat16
    f8 = mybir.dt.float8e4
    Alu = mybir.AluOpType
    ActFn = mybir.ActivationFunctionType
    DR = mybir.MatmulPerfMode.DoubleRow

    nc = bacc.Bacc("TRN2", target_bir_lowering=False, debug=False)

    xm_d = nc.declare_dram_parameter("xm", [NBLK, 128, KP, 2, NB], f8, isOutput=False)
    dxm_d = nc.declare_dram_parameter("dxm", [NBLK, 128, KP, 2, NB], f8, isOutput=False)
    hm_d = nc.declare_dram_parameter("hm", [NBLK, 128, KP, 2, NB], f8, isOutput=False)
    hb_d = nc.declare_dram_parameter("hb", [NBLK, 128, NJ, NB], bf16, isOutput=False)
    wih_d = nc.declare_dram_parameter("wihT", [128, NJ, 3, KP, 2, 128], f8, isOutput=False)
    whh_d = nc.declare_dram_parameter("whhT", [128, NJ, 3, KP, 2, 128], f8, isOutput=False)
    dw_d = nc.declare_dram_parameter("dwT", [128, NJ, KP, 2, 128], f8, isOutput=False)
    bias_d = nc.declare_dram_parameter("biasT", [128, NJ, 4], f32, isOutput=False)
    out_d = nc.declare_dram_parameter("h_next", [NBLK, 128, NJ, NB], bf16, isOutput=True)

    with tile.TileContext(nc) as tc:
        with (
            tc.tile_pool(name="wpool", bufs=1) as wpool,
            tc.tile_pool(name="stream", bufs=2) as stream,
            tc.tile_pool(name="temps", bufs=3) as temps,
            tc.tile_pool(name="psum", bufs=2, space="PSUM") as psum,
        ):
            wih_t = wpool.tile([128, NJ, 3, KP, 2, 128], f8, tag="wih")
            whh_t = wpool.tile([128, NJ, 3, KP, 2, 128], f8, tag="whh")
            dw_t = wpool.tile([128, NJ, KP, 2, 128], f8, tag="dw")
            bias_t = wpool.tile([128, NJ, 4], f32, tag="bias")

            # Weights in consumption order, split across the two DMA paths so
            # the first j-chunks land fast.
            nc.sync.dma_start(out=bias_t[:], in_=bias_d[:])
            for j in range(NJ):
                nc.sync.dma_start(out=wih_t[:, j], in_=wih_d[:, j])
                nc.gpsimd.dma_start(out=whh_t[:, j], in_=whh_d[:, j])
                nc.gpsimd.dma_start(out=dw_t[:, j], in_=dw_d[:, j])

            for bb in range(NBLK):
                xm_t = stream.tile([128, KP, 2, NB], f8, tag="xm")
                hm_t = stream.tile([128, KP, 2, NB], f8, tag="hm")
                dxm_t = stream.tile([128, KP, 2, NB], f8, tag="dxm")
                hb_t = stream.tile([128, NJ, NB], bf16, tag="hb")
                out_t = stream.tile([128, NJ, NB], bf16, tag="out")
                nc.sync.dma_start(out=xm_t[:], in_=xm_d[bb])
                nc.sync.dma_start(out=hm_t[:], in_=hm_d[bb])
                nc.sync.dma_start(out=dxm_t[:], in_=dxm_d[bb])
                nc.sync.dma_start(out=hb_t[:], in_=hb_d[bb])

                for j in range(NJ):
                    R = psum.tile([128, NB], f32, tag="R")
                    Z = psum.tile([128, NB], f32, tag="Z")
                    NI = psum.tile([128, NB], f32, tag="NI")
                    NH = psum.tile([128, NB], f32, tag="NH")

                    # r/z gates: x-side + h-side accumulate into one bank
                    for g, P_ in ((0, R), (1, Z)):
                        for kp in range(KP):
                            nc.tensor.matmul(P_[:], wih_t[:, j, g, kp], xm_t[:, kp],
                                             start=(kp == 0), stop=False, perf_mode=DR)
                        for kp in range(KP):
                            nc.tensor.matmul(P_[:], whh_t[:, j, g, kp], hm_t[:, kp],
                                             start=False, stop=(kp == KP - 1), perf_mode=DR)
                    # n gate, h-side
                    for kp in range(KP):
                        nc.tensor.matmul(NH[:], whh_t[:, j, 2, kp], hm_t[:, kp],
                                         start=(kp == 0), stop=(kp == KP - 1), perf_mode=DR)
                    # n gate, x-side with fp8 residual compensation
                    for kp in range(KP):
                        nc.tensor.matmul(NI[:], wih_t[:, j, 2, kp], xm_t[:, kp],
                                         start=(kp == 0), stop=False, perf_mode=DR)
                    for kp in range(KP):
                        nc.tensor.matmul(NI[:], wih_t[:, j, 2, kp], dxm_t[:, kp],
                                         start=False, stop=False, perf_mode=DR)
                    for kp in range(KP):
                        nc.tensor.matmul(NI[:], dw_t[:, j, kp], xm_t[:, kp],
                                         start=False, stop=(kp == KP - 1), perf_mode=DR)

                    # epilogue
                    r = temps.tile([128, NB], bf16, tag="r")
                    nc.scalar.activation(r[:], R[:], ActFn.Sigmoid,
                                         bias=bias_t[:, j, 0:1], scale=1.0 / S)
                    z = temps.tile([128, NB], bf16, tag="z")
                    nc.scalar.activation(z[:], Z[:], ActFn.Sigmoid,
                                         bias=bias_t[:, j, 1:2], scale=1.0 / S)
                    # t = (NH + 32*bnh) * r
                    t = temps.tile([128, NB], bf16, tag="t")
                    nc.vector.scalar_tensor_tensor(t[:], NH[:], bias_t[:, j, 3:4], r[:],
                                                   Alu.add, Alu.mult)
                    # NI += t (in place, PSUM)
                    nc.vector.tensor_tensor(NI[:], NI[:], t[:], Alu.add)
                    n = temps.tile([128, NB], bf16, tag="n")
                    nc.scalar.activation(n[:], NI[:], ActFn.Tanh,
                                         bias=bias_t[:, j, 2:3], scale=1.0 / S)
                    # h' = n + z*(h - n)
                    hm1 = temps.tile([128, NB], bf16, tag="hm1")
                    nc.vector.tensor_tensor(hm1[:], hb_t[:, j], n[:], Alu.subtract)
                    hm2 = temps.tile([128, NB], bf16, tag="hm2")
                    nc.gpsimd.tensor_tensor(hm2[:], z[:], hm1[:], Alu.mult)
                    nc.vector.tensor_tensor(out_t[:, j], n[:], hm2[:], Alu.add)
                    nc.gpsimd.dma_start(out=out_d[bb, :, j], in_=out_t[:, j])

    nc.compile()
    return nc


def _prep_inputs(x, h_prev, weight_ih, weight_hh, bias_ih, bias_hh):
    import ml_dtypes

    bf16 = ml_dtypes.bfloat16
    f8 = ml_dtypes.float8_e4m3fn if hasattr(ml_dtypes, "float8_e4m3fn") else ml_dtypes.float8_e4m3

    def q8(a):
        return a.astype(f8)

    xq = q8(x)
    hq = q8(h_prev)
    dx = q8(x - xq.astype(np.float32))

    # moving operands: [core, blk, p, kp, i, b] = a[core*2048+blk*512+b, (2kp+i)*128+p]
    def to_moving(a8):
        v = a8.reshape(NCORES, NBLK, NB, KP, 2, 128).transpose(0, 1, 5, 3, 4, 2)
        return np.ascontiguousarray(v)

    xm = to_moving(xq)
    hm = to_moving(hq)
    dxm = to_moving(dx)
    hb = np.ascontiguousarray(
        h_prev.reshape(NCORES, NBLK, NB, NJ, 128).transpose(0, 1, 4, 3, 2)
    ).astype(bf16)

    # stationary weights: [pk, j, g, kp, i, mg] = Wq[g*1024 + j*128 + mg, (2kp+i)*128 + pk]
    def to_stationary(w8):
        v = w8.reshape(3, NJ, 128, KP, 2, 128).transpose(5, 1, 0, 3, 4, 2)
        return np.ascontiguousarray(v)

    wihq = q8(S * weight_ih)
    whhq = q8(S * weight_hh)
    wihT = to_stationary(wihq)
    whhT = to_stationary(whhq)
    dwn = q8(S * weight_ih[2 * H:] - wihq[2 * H:].astype(np.float32))
    dwT = np.ascontiguousarray(
        dwn.reshape(NJ, 128, KP, 2, 128).transpose(4, 0, 2, 3, 1)
    )

    bias = np.empty((128, NJ, 4), np.float32)
    br = (bias_ih[:H] + bias_hh[:H]).reshape(NJ, 128)
    bz = (bias_ih[H:2 * H] + bias_hh[H:2 * H]).reshape(NJ, 128)
    bni = bias_ih[2 * H:].reshape(NJ, 128)
    bnh = (S * bias_hh[2 * H:]).reshape(NJ, 128)
    bias[:, :, 0] = br.T
    bias[:, :, 1] = bz.T
    bias[:, :, 2] = bni.T
    bias[:, :, 3] = bnh.T

    in_maps = []
    for c in range(NCORES):
        in_maps.append({
            "xm": xm[c], "dxm": dxm[c], "hm": hm[c], "hb": hb[c],
            "wihT": wihT, "whhT": whhT, "dwT": dwT, "biasT": bias,
        })
    return in_maps


def kernel(x, h_prev, weight_ih, weight_hh, bias_ih, bias_hh):
    from concourse.bass_utils import run_bass_kernel_spmd

    x = np.asarray(x, dtype=np.float32)
    h_prev = np.asarray(h_prev, dtype=np.float32)
    weight_ih = np.asarray(weight_ih, dtype=np.float32)
    weight_hh = np.asarray(weight_hh, dtype=np.float32)
    bias_ih = np.asarray(bias_ih, dtype=np.float32)
    bias_hh = np.asarray(bias_hh, dtype=np.float32)

    if "nc" not in _cache:
        _cache["nc"] = _build_program()
    nc = _cache["nc"]

    in_maps = _prep_inputs(x, h_prev, weight_ih, weight_hh, bias_ih, bias_hh)
    trace = os.environ.get("GRU_TRACE", "0") == "1"
    res = run_bass_kernel_spmd(nc, in_maps, list(range(NCORES)), trace=trace)
    kernel._last_exec_ns = res.exec_time_ns

    outs = []
    for c in range(NCORES):
        o = np.asarray(res.results[c]["h_next"])  # [NBLK, 128, NJ, NB] bf16
        outs.append(o.transpose(0, 3, 2, 1).reshape(BLOC, H))
    return np.concatenate(outs, axis=0).astype(np.float32)


kernel._last_exec_ns = None


# revision 4
# speedup vs baseline: 1.7169x; 1.1772x over previous
"""GRU single-step kernel for Trainium2, data-parallel over 8 NeuronCores.

Computes h_next = GRUCell(x, h_prev) with PyTorch gate layout [r; z; n]:
    gi = x @ W_ih.T + b_ih ; gh = h @ W_hh.T + b_hh
    r = sigmoid(gi_r + gh_r); z = sigmoid(gi_z + gh_z)
    n = tanh(gi_n + r * gh_n); h' = (1-z)*n + z*h

Strategy: shard batch (16384 -> 8 x 2048); weights replicated. Weight-
stationary layout with gates on PSUM partitions and batch on the free dim, so
the per-gate biases fold into the ACT engine's per-partition bias operand.
Matmul dtype per piece chosen by error sensitivity (hw fp8 DoubleRow = 2x
bf16 MAC rate; a DR pass covers 256 contraction rows vs bf16's 128):
  fp8(e4m3)+DoubleRow: r/z both sides, n h-side   (insensitive pieces)
  bf16:                n-gate x-side              (dominant error term)
Host-side sim of this mix: rel err 1.40e-2 (gate is 2e-2). Weights are
pre-scaled by 32 so fp8 stays out of subnormals; 1/32 folds into ACT scale.

Epilogue per (block, j) on [128 gates, 512 batch] tiles:
  ACT : r = sig(R/32 + br), z = sig(Z/32 + bz), n = tanh(NI'/32 + bni)
  DVE : t = (NH + 32*bnh)*r  (fused scalar_tensor_tensor), NI' = NI + t
        (in-place PSUM), hm1 = h - n, out = n + hm2
  POOL: hm2 = z*hm1, input h + output DMA issue (cheap SWDGE)
All sigmoid/tanh live in one ACT table -> no table reloads.

DMA schedule: three queues (SP, Pool-SWDGE, ACT) with the j=0 weights and
block-0 operands first so the PE starts ~10us in; weight j-chunks stream in
consumption order behind them.
"""

import os
import sys

import numpy as np

if "/opt/trn_rl_repo" not in sys.path:
    sys.path.insert(0, "/opt/trn_rl_repo")

H = 1024            # hidden == input size
B = 16384
NCORES = 8
BLOC = B // NCORES  # 2048 rows per core
NB = 512            # batch columns per block (PSUM bank width)
NBLK = BLOC // NB   # 4 blocks per core
KP = 4              # fp8 DoubleRow k-pairs (2x128 contraction per pass)
KC = 8              # bf16 k-chunks (128 contraction per pass)
NJ = H // 128       # 8 hidden chunks of 128 gates
S = 32.0            # weight pre-scale

_cache = {}


def _build_program():
    from concourse import bacc, bass, mybir, tile

    f32 = mybir.dt.float32
    bf16 = mybir.dt.bfloat16
    f8 = mybir.dt.float8e4
    Alu = mybir.AluOpType
    ActFn = mybir.ActivationFunctionType
    DR = mybir.MatmulPerfMode.DoubleRow

    nc = bacc.Bacc("TRN2", target_bir_lowering=False, debug=False)

    xm_d = nc.declare_dram_parameter("xm", [NBLK, 128, KP, 2, NB], f8, isOutput=False)
    xb_d = nc.declare_dram_parameter("xb", [NBLK, 128, KC, NB], bf16, isOutput=False)
    hm_d = nc.declare_dram_parameter("hm", [NBLK, 128, KP, 2, NB], f8, isOutput=False)
    hb_d = nc.declare_dram_parameter("hb", [NBLK, 128, NJ, NB], bf16, isOutput=False)
    wih_d = nc.declare_dram_parameter("wihT", [128, NJ, 2, KP, 2, 128], f8, isOutput=False)
    win_d = nc.declare_dram_parameter("winT", [128, NJ, KC, 128], bf16, isOutput=False)
    whh_d = nc.declare_dram_parameter("whhT", [128, NJ, 3, KP, 2, 128], f8, isOutput=False)
    bias_d = nc.declare_dram_parameter("biasT", [128, NJ, 4], f32, isOutput=False)
    out_d = nc.declare_dram_parameter("h_next", [NBLK, 128, NJ, NB], bf16, isOutput=True)

    with tile.TileContext(nc) as tc:
        with (
            tc.tile_pool(name="wpool", bufs=1) as wpool,
            tc.tile_pool(name="stream", bufs=2) as stream,
            tc.tile_pool(name="temps", bufs=3) as temps,
            tc.tile_pool(name="psum", bufs=2, space="PSUM") as psum,
        ):
            wih_t = wpool.tile([128, NJ, 2, KP, 2, 128], f8, tag="wih")
            win_t = wpool.tile([128, NJ, KC, 128], bf16, tag="win")
            whh_t = wpool.tile([128, NJ, 3, KP, 2, 128], f8, tag="whh")
            bias_t = wpool.tile([128, NJ, 4], f32, tag="bias")

            xm_ts, xb_ts, hm_ts, hb_ts = [], [], [], []
            for bb in range(NBLK):
                xm_ts.append(stream.tile([128, KP, 2, NB], f8, tag="xm", name=f"xm{bb}"))
                xb_ts.append(stream.tile([128, KC, NB], bf16, tag="xb", name=f"xb{bb}"))
                hm_ts.append(stream.tile([128, KP, 2, NB], f8, tag="hm", name=f"hm{bb}"))
                hb_ts.append(stream.tile([128, NJ, NB], bf16, tag="hb", name=f"hb{bb}"))

            # Startup: j=0 weights + block-0/1 operands first, then the rest
            # of the weights in consumption order. Three parallel queues.
            nc.sync.dma_start(out=bias_t[:], in_=bias_d[:])
            nc.sync.dma_start(out=wih_t[:, 0], in_=wih_d[:, 0])
            nc.sync.dma_start(out=xm_ts[0][:], in_=xm_d[0])
            nc.gpsimd.dma_start(out=whh_t[:, 0], in_=whh_d[:, 0])
            nc.gpsimd.dma_start(out=hm_ts[0][:], in_=hm_d[0])
            nc.scalar.dma_start(out=xb_ts[0][:], in_=xb_d[0])
            nc.scalar.dma_start(out=hb_ts[0][:], in_=hb_d[0])
            nc.sync.dma_start(out=win_t[:, 0], in_=win_d[:, 0])
            for j in range(1, NJ):
                nc.sync.dma_start(out=wih_t[:, j], in_=wih_d[:, j])
                nc.sync.dma_start(out=win_t[:, j], in_=win_d[:, j])
                nc.gpsimd.dma_start(out=whh_t[:, j], in_=whh_d[:, j])
            nc.sync.dma_start(out=xm_ts[1][:], in_=xm_d[1])
            nc.gpsimd.dma_start(out=hm_ts[1][:], in_=hm_d[1])
            nc.scalar.dma_start(out=xb_ts[1][:], in_=xb_d[1])
            nc.scalar.dma_start(out=hb_ts[1][:], in_=hb_d[1])

            for bb in range(NBLK):
                xm_t, xb_t, hm_t, hb_t = xm_ts[bb], xb_ts[bb], hm_ts[bb], hb_ts[bb]
                if bb >= 2:
                    nc.sync.dma_start(out=xm_t[:], in_=xm_d[bb])
                    nc.gpsimd.dma_start(out=hm_t[:], in_=hm_d[bb])
                    nc.scalar.dma_start(out=xb_t[:], in_=xb_d[bb])
                    nc.scalar.dma_start(out=hb_t[:], in_=hb_d[bb])
                out_t = stream.tile([128, NJ, NB], bf16, tag="out")

                for j in range(NJ):
                    R = psum.tile([128, NB], f32, tag="R")
                    Z = psum.tile([128, NB], f32, tag="Z")
                    NI = psum.tile([128, NB], f32, tag="NI")
                    NH = psum.tile([128, NB], f32, tag="NH")

                    # r/z gates: fp8 DR, x-side + h-side into one bank
                    for g, P_ in ((0, R), (1, Z)):
                        for kp in range(KP):
                            nc.tensor.matmul(P_[:], wih_t[:, j, g, kp], xm_t[:, kp],
                                             start=(kp == 0), stop=False, perf_mode=DR)
                        for kp in range(KP):
                            nc.tensor.matmul(P_[:], whh_t[:, j, g, kp], hm_t[:, kp],
                                             start=False, stop=(kp == KP - 1), perf_mode=DR)
                    # n gate, h-side: fp8 DR
                    for kp in range(KP):
                        nc.tensor.matmul(NH[:], whh_t[:, j, 2, kp], hm_t[:, kp],
                                         start=(kp == 0), stop=(kp == KP - 1), perf_mode=DR)
                    # n gate, x-side: bf16 (dominant error term kept exact)
                    for kc in range(KC):
                        nc.tensor.matmul(NI[:], win_t[:, j, kc], xb_t[:, kc],
                                         start=(kc == 0), stop=(kc == KC - 1))

                    # epilogue
                    r = temps.tile([128, NB], bf16, tag="r")
                    nc.scalar.activation(r[:], R[:], ActFn.Sigmoid,
                                         bias=bias_t[:, j, 0:1], scale=1.0 / S)
                    z = temps.tile([128, NB], bf16, tag="z")
                    nc.scalar.activation(z[:], Z[:], ActFn.Sigmoid,
                                         bias=bias_t[:, j, 1:2], scale=1.0 / S)
                    # t = (NH + 32*bnh) * r
                    t = temps.tile([128, NB], bf16, tag="t")
                    nc.vector.scalar_tensor_tensor(t[:], NH[:], bias_t[:, j, 3:4], r[:],
                                                   Alu.add, Alu.mult)
                    # NI += t (in place, PSUM)
                    nc.vector.tensor_tensor(NI[:], NI[:], t[:], Alu.add)
                    n = temps.tile([128, NB], bf16, tag="n")
                    nc.scalar.activation(n[:], NI[:], ActFn.Tanh,
                                         bias=bias_t[:, j, 2:3], scale=1.0 / S)
                    # h' = n + z*(h - n)
                    hm1 = temps.tile([128, NB], bf16, tag="hm1")
                    nc.vector.tensor_tensor(hm1[:], hb_t[:, j], n[:], Alu.subtract)
                    hm2 = temps.tile([128, NB], bf16, tag="hm2")
                    nc.gpsimd.tensor_tensor(hm2[:], z[:], hm1[:], Alu.mult)
                    nc.vector.tensor_tensor(out_t[:, j], n[:], hm2[:], Alu.add)
                    nc.gpsimd.dma_start(out=out_d[bb, :, j], in_=out_t[:, j])

    nc.compile()
    return nc


def _prep_inputs(x, h_prev, weight_ih, weight_hh, bias_ih, bias_hh):
    import ml_dtypes

    bf16 = ml_dtypes.bfloat16
    f8 = ml_dtypes.float8_e4m3fn if hasattr(ml_dtypes, "float8_e4m3fn") else ml_dtypes.float8_e4m3

    # fp8 moving: [core, blk, p, kp, i, b] = a[core*2048+blk*512+b, (2kp+i)*128+p]
    def to_moving8(a):
        v = a.astype(f8).reshape(NCORES, NBLK, NB, KP, 2, 128).transpose(0, 1, 5, 3, 4, 2)
        return np.ascontiguousarray(v)

    xm = to_moving8(x)
    hm = to_moving8(h_prev)
    # bf16 moving: [core, blk, p, kc, b] = x[core*2048+blk*512+b, kc*128+p]
    xb = np.ascontiguousarray(
        x.astype(bf16).reshape(NCORES, NBLK, NB, KC, 128).transpose(0, 1, 4, 3, 2))
    hb = np.ascontiguousarray(
        h_prev.astype(bf16).reshape(NCORES, NBLK, NB, NJ, 128).transpose(0, 1, 4, 3, 2))

    # fp8 stationary: [pk, j, g, kp, i, mg] = Wq[g*1024 + j*128 + mg, (2kp+i)*128 + pk]
    def to_stationary8(w, ngates):
        v = (S * w).astype(f8).reshape(ngates, NJ, 128, KP, 2, 128).transpose(5, 1, 0, 3, 4, 2)
        return np.ascontiguousarray(v)

    wihT = to_stationary8(weight_ih[:2 * H], 2)
    whhT = to_stationary8(weight_hh, 3)
    # bf16 stationary n-gate x-side: [pk, j, kc, mg] = 32*Wih_n[j*128+mg, kc*128+pk]
    winT = np.ascontiguousarray(
        (S * weight_ih[2 * H:]).astype(bf16).reshape(NJ, 128, KC, 128).transpose(3, 0, 2, 1))

    bias = np.empty((128, NJ, 4), np.float32)
    bias[:, :, 0] = (bias_ih[:H] + bias_hh[:H]).reshape(NJ, 128).T
    bias[:, :, 1] = (bias_ih[H:2 * H] + bias_hh[H:2 * H]).reshape(NJ, 128).T
    bias[:, :, 2] = bias_ih[2 * H:].reshape(NJ, 128).T
    bias[:, :, 3] = (S * bias_hh[2 * H:]).reshape(NJ, 128).T

    in_maps = []
    for c in range(NCORES):
        in_maps.append({
            "xm": xm[c], "xb": xb[c], "hm": hm[c], "hb": hb[c],
            "wihT": wihT, "winT": winT, "whhT": whhT, "biasT": bias,
        })
    return in_maps


def kernel(x, h_prev, weight_ih, weight_hh, bias_ih, bias_hh):
    from concourse.bass_utils import run_bass_kernel_spmd

    x = np.asarray(x, dtype=np.float32)
    h_prev = np.asarray(h_prev, dtype=np.float32)
    weight_ih = np.asarray(weight_ih, dtype=np.float32)
    weight_hh = np.asarray(weight_hh, dtype=np.float32)
    bias_ih = np.asarray(bias_ih, dtype=np.float32)
    bias_hh = np.asarray(bias_hh, dtype=np.float32)

    if "nc" not in _cache:
        _cache["nc"] = _build_program()
    nc = _cache["nc"]

    in_maps = _prep_inputs(x, h_prev, weight_ih, weight_hh, bias_ih, bias_hh)
    trace = os.environ.get("GRU_TRACE", "0") == "1"
    res = run_bass_kernel_spmd(nc, in_maps, list(range(NCORES)), trace=trace)
    kernel._last_exec_ns = res.exec_time_ns

    outs = []
    for c in range(NCORES):
        o = np.asarray(res.results[c]["h_next"])  # [NBLK, 128, NJ, NB] bf16
        outs.append(o.transpose(0, 3, 2, 1).reshape(BLOC, H))
    return np.concatenate(outs, axis=0).astype(np.float32)


kernel._last_exec_ns = None
